# revision 30
# baseline (speedup 1.0000x reference)
"""Anchor3DHead conv heads (three 1x1 convs) as a data-parallel Bass kernel.

Reference computes, for x [B=4, C=384, H=248, W=216]:
    cls  = einsum('bchw,oc->bohw', x, w_cls) + b_cls   # O=18
    reg  = einsum('bchw,oc->bohw', x, w_reg) + b_reg   # O=42
    dir  = einsum('bchw,oc->bohw', x, w_dir) + b_dir   # O=12

All three heads read the same x, so they are fused into a single GEMM with
O=72 output channels.  x[b] is already a contiguous [C, H*W] matrix, so the
whole problem is out[72, P] = W[72, 384] @ x[384, P] + b over P = B*H*W
pixels.  Sharding: pure data parallel over pixels — each of the 8 cores gets
half of one batch element ([384, 26784]); weights/bias are replicated.

The problem is memory-bound (headroom target_regime=memory): per-core HBM
traffic dominates, so input bytes are the whole game.  x is staged to the
device as fp8 E3M4 (TRN2-native FP8_EXP3, 4 mantissa bits); the TensorEngine
runs a mixed-dtype matmul (bf16 stationary weights x e3m4 moving operand —
verified bit-exact on HW) with fp32 PSUM accumulation.  The output is stored
bf16 and widened to f32 on the host.  Max rel err vs the f32 reference on
the real inputs is 1.46e-2, under the 2e-2 gate (bf16-x fallback: 4e-3).
"""

import os

import numpy as np

B, C, H, W = 4, 384, 248, 216
HWP = H * W            # 53568 pixels per batch element
HALF = HWP // 2        # 26784 pixels per core
O_CLS, O_REG, O_DIR = 18, 42, 12
O = O_CLS + O_REG + O_DIR  # 72 fused output channels
NCORES = 8
KT = C // 128          # 3 contraction tiles of 128
SUB = 496              # matmul free dim (<=512 fp32, 496*4B fits one PSUM bank)
NSUB = 6               # psum tiles per chunk
CHUNK = SUB * NSUB     # 2976 columns per DMA chunk
NCHUNK = HALF // CHUNK  # 9 chunks per core

LAST_RESULT = None  # BassKernelResults of the most recent run (for test.py)

_nc_cache = None


def _build_nc(
    repeats=1,
    xp_bufs=12,
    op_bufs=4,
    pp_bufs=8,
    store_split=1,
    dve_bias_frac=2,
    k_outer=False,
):
    import concourse.bacc as bacc
    import concourse.mybir as mybir
    import concourse.tile as tile

    nc = bacc.Bacc(
        "TRN2", target_bir_lowering=False, debug=False, num_devices=NCORES
    )
    fp32 = mybir.dt.float32
    bf16 = mybir.dt.bfloat16
    e3m4 = mybir.dt.float8e3
    x_d = nc.declare_dram_parameter("x", [C, HALF], e3m4, isOutput=False)
    w_d = nc.declare_dram_parameter("w", [C, O], bf16, isOutput=False)
    b_d = nc.declare_dram_parameter("b", [O, 1], fp32, isOutput=False)
    out_d = nc.declare_dram_parameter("out", [O, HALF], bf16, isOutput=True)

    # Column segments: full chunks first, then a finer-grained tail so the
    # last load->matmul->store dependency chain is short.
    segs = []
    col = 0
    for _ in range(NCHUNK - 1):
        segs.append((col, NSUB))
        col += CHUNK
    for nsub in (2, 2, 2):
        segs.append((col, nsub))
        col += nsub * SUB
    assert col == HALF
    with tile.TileContext(nc) as tc:
        with (
            tc.tile_pool(name="wp", bufs=1) as wp,
            tc.tile_pool(name="xp", bufs=xp_bufs) as xp,
            tc.tile_pool(name="op", bufs=op_bufs) as op,
            tc.tile_pool(name="pp", bufs=pp_bufs, space="PSUM") as pp,
        ):
            # Stationary operand: w is already [K, M] = [384, 72] (lhsT).
            w_t = wp.tile([128, KT * O], bf16)
            b_t = wp.tile([O, 1], fp32)

            def load_seg(col0, nsub):
                width = nsub * SUB
                xts = []
                for k in range(KT):
                    xt = xp.tile([128, width], e3m4, tag="x")
                    nc.sync.dma_start(
                        xt[:], x_d[k * 128 : (k + 1) * 128, col0 : col0 + width]
                    )
                    xts.append(xt)
                return xts

            # first big loads go ahead of the tiny w/bias DMAs so the DMA
            # engines ramp to full rate immediately
            pending = load_seg(*segs[0])
            for k in range(KT):
                nc.sync.dma_start(
                    w_t[:, k * O : (k + 1) * O], w_d[k * 128 : (k + 1) * 128, :]
                )
            nc.sync.dma_start(b_t[:], b_d[:])

            def bias_add(o_slice, ps, s):
                # out = psum + bias (per-partition), PSUM -> SBUF; spread the
                # work across ACT and the otherwise-idle DVE
                if dve_bias_frac and (s % dve_bias_frac) == 0:
                    nc.vector.tensor_scalar_add(o_slice, ps[:], b_t[:])
                else:
                    nc.scalar.add(o_slice, ps[:], b_t[:])

            def compute_seg(col0, nsub, xts):
                width = nsub * SUB
                o_t = op.tile([O, width], bf16, tag="o")
                per_store = max(1, nsub // store_split)
                if k_outer:
                    pss = [
                        pp.tile([O, SUB], fp32, tag="ps", name=f"ps_{col0}_{i}")
                        for i in range(nsub)
                    ]
                    for k in range(KT):
                        for s in range(nsub):
                            nc.tensor.matmul(
                                pss[s][:],
                                w_t[:, k * O : (k + 1) * O],
                                xts[k][:, s * SUB : (s + 1) * SUB],
                                start=(k == 0),
                                stop=(k == KT - 1),
                            )
                    for s in range(nsub):
                        bias_add(o_t[:, s * SUB : (s + 1) * SUB], pss[s], s)
                    nc.scalar.dma_start(out_d[:, col0 : col0 + width], o_t[:])
                    return
                stored = 0
                for s in range(nsub):
                    ps = pp.tile([O, SUB], fp32, tag="ps")
                    for k in range(KT):
                        nc.tensor.matmul(
                            ps[:],
                            w_t[:, k * O : (k + 1) * O],
                            xts[k][:, s * SUB : (s + 1) * SUB],
                            start=(k == 0),
                            stop=(k == KT - 1),
                        )
                    bias_add(o_t[:, s * SUB : (s + 1) * SUB], ps, s)
                    # store on the ACT HWDGE ring; loads use the SP ring
                    if s + 1 - stored >= per_store or s + 1 == nsub:
                        nc.scalar.dma_start(
                            out_d[:, col0 + stored * SUB : col0 + (s + 1) * SUB],
                            o_t[:, stored * SUB : (s + 1) * SUB],
                        )
                        stored = s + 1

            if repeats == 1:
                for si, (col0, nsub) in enumerate(segs):
                    xts = pending
                    if si + 1 < len(segs):
                        pending = load_seg(*segs[si + 1])
                    compute_seg(col0, nsub, xts)
            else:
                # hardware-loop variant for slope-based HW timing only
                compute_seg(*segs[0], pending)
                with tc.For_i(0, repeats, 1, hint_engines=(
                    mybir.EngineType.PE,
                    mybir.EngineType.Activation,
                    mybir.EngineType.SP,
                    mybir.EngineType.DVE,
                )):
                    for col0, nsub in segs:
                        xts = load_seg(col0, nsub)
                        compute_seg(col0, nsub, xts)
    nc.compile()
    return nc


def kernel(x, w_cls, b_cls, w_reg, b_reg, w_dir, b_dir):
    global LAST_RESULT, _nc_cache
    import ml_dtypes
    from concourse.bass_utils import run_bass_kernel_spmd

    bf = ml_dtypes.bfloat16
    e3 = ml_dtypes.float8_e3m4
    x = np.asarray(x, dtype=np.float32)
    w_all = np.concatenate(
        [np.asarray(w_cls), np.asarray(w_reg), np.asarray(w_dir)], axis=0
    ).astype(np.float32)  # [72, 384]
    lhsT = np.ascontiguousarray(w_all.T).astype(bf)  # [384, 72] bf16
    bias = (
        np.concatenate([np.asarray(b_cls), np.asarray(b_reg), np.asarray(b_dir)])
        .astype(np.float32)
        .reshape(O, 1)
    )

    xf = x.reshape(B, C, HWP)
    in_maps = []
    for core in range(NCORES):
        b_i, h_i = divmod(core, 2)
        shard = xf[b_i, :, h_i * HALF : (h_i + 1) * HALF].astype(e3)
        in_maps.append({"x": shard, "w": lhsT, "b": bias})

    if _nc_cache is None:
        _nc_cache = _build_nc()
    trace = os.environ.get("ANCHOR3D_TRACE", "0") == "1"
    LAST_RESULT = run_bass_kernel_spmd(
        _nc_cache, in_maps, core_ids=list(range(NCORES)), trace=trace
    )
    res = LAST_RESULT.results

    out = np.empty((B, O, HWP), dtype=np.float32)
    for core in range(NCORES):
        b_i, h_i = divmod(core, 2)
        out[b_i, :, h_i * HALF : (h_i + 1) * HALF] = res[core]["out"].astype(
            np.float32
        )
    out = out.reshape(B, O, H, W)
    return (
        np.ascontiguousarray(out[:, :O_CLS]),
        np.ascontiguousarray(out[:, O_CLS : O_CLS + O_REG]),
        np.ascontiguousarray(out[:, O_CLS + O_REG :]),
    )


# revision 32
# speedup vs baseline: 1.0173x; 1.0173x over previous
"""Anchor3DHead conv heads (three 1x1 convs) as a data-parallel Bass kernel.

Reference computes, for x [B=4, C=384, H=248, W=216]:
    cls  = einsum('bchw,oc->bohw', x, w_cls) + b_cls   # O=18
    reg  = einsum('bchw,oc->bohw', x, w_reg) + b_reg   # O=42
    dir  = einsum('bchw,oc->bohw', x, w_dir) + b_dir   # O=12

All three heads read the same x, so they are fused into a single GEMM with
O=72 output channels.  x[b] is already a contiguous [C, H*W] matrix, so the
whole problem is out[72, P] = W[72, 384] @ x[384, P] + b over P = B*H*W
pixels.  Sharding: pure data parallel over pixels — each of the 8 cores gets
half of one batch element ([384, 26784]); weights/bias are replicated.

The problem is memory-bound (headroom target_regime=memory): per-core HBM
traffic dominates, so input bytes are the whole game.  x is staged to the
device as fp8 E3M4 (TRN2-native FP8_EXP3, 4 mantissa bits); the TensorEngine
runs a mixed-dtype matmul (bf16 stationary weights x e3m4 moving operand —
verified bit-exact on HW) with fp32 PSUM accumulation.  The output is stored
bf16 and widened to f32 on the host.  Max rel err vs the f32 reference on
the real inputs is 1.46e-2, under the 2e-2 gate (bf16-x fallback: 4e-3).
"""

import os

import numpy as np

B, C, H, W = 4, 384, 248, 216
HWP = H * W            # 53568 pixels per batch element
HALF = HWP // 2        # 26784 pixels per core
O_CLS, O_REG, O_DIR = 18, 42, 12
O = O_CLS + O_REG + O_DIR  # 72 fused output channels
NCORES = 8
KT = C // 128          # 3 contraction tiles of 128
SUB = 496              # matmul free dim (<=512 fp32, 496*4B fits one PSUM bank)
NSUB = 4               # psum tiles per chunk
CHUNK = SUB * NSUB     # 2976 columns per DMA chunk
NCHUNK = HALF // CHUNK  # 9 chunks per core

LAST_RESULT = None  # BassKernelResults of the most recent run (for test.py)

_nc_cache = None


def _build_nc(
    repeats=1,
    xp_bufs=18,
    op_bufs=4,
    pp_bufs=8,
    store_split=1,
    dve_bias_frac=2,
    k_outer=False,
):
    import concourse.bacc as bacc
    import concourse.mybir as mybir
    import concourse.tile as tile

    nc = bacc.Bacc(
        "TRN2", target_bir_lowering=False, debug=False, num_devices=NCORES
    )
    fp32 = mybir.dt.float32
    bf16 = mybir.dt.bfloat16
    e3m4 = mybir.dt.float8e3
    x_d = nc.declare_dram_parameter("x", [C, HALF], e3m4, isOutput=False)
    w_d = nc.declare_dram_parameter("w", [C, O], bf16, isOutput=False)
    b_d = nc.declare_dram_parameter("b", [O, 1], fp32, isOutput=False)
    out_d = nc.declare_dram_parameter("out", [O, HALF], bf16, isOutput=True)

    # Column segments: full chunks first, then a finer-grained tail so the
    # last load->matmul->store dependency chain is short.
    segs = []
    col = 0
    for _ in range(NCHUNK - 1):
        segs.append((col, NSUB))
        col += CHUNK
    for nsub in (2, 2, 2):
        segs.append((col, nsub))
        col += nsub * SUB
    assert col == HALF
    with tile.TileContext(nc) as tc:
        with (
            tc.tile_pool(name="wp", bufs=1) as wp,
            tc.tile_pool(name="xp", bufs=xp_bufs) as xp,
            tc.tile_pool(name="op", bufs=op_bufs) as op,
            tc.tile_pool(name="pp", bufs=pp_bufs, space="PSUM") as pp,
        ):
            # Stationary operand: w is already [K, M] = [384, 72] (lhsT).
            w_t = wp.tile([128, KT * O], bf16)
            b_t = wp.tile([O, 1], fp32)

            def load_seg(col0, nsub):
                width = nsub * SUB
                xts = []
                for k in range(KT):
                    xt = xp.tile([128, width], e3m4, tag="x")
                    nc.sync.dma_start(
                        xt[:], x_d[k * 128 : (k + 1) * 128, col0 : col0 + width]
                    )
                    xts.append(xt)
                return xts

            # first big loads go ahead of the tiny w/bias DMAs so the DMA
            # engines ramp to full rate immediately
            pending = load_seg(*segs[0])
            for k in range(KT):
                nc.sync.dma_start(
                    w_t[:, k * O : (k + 1) * O], w_d[k * 128 : (k + 1) * 128, :]
                )
            nc.sync.dma_start(b_t[:], b_d[:])

            def bias_add(o_slice, ps, s):
                # out = psum + bias (per-partition), PSUM -> SBUF; spread the
                # work across ACT and the otherwise-idle DVE
                if dve_bias_frac and (s % dve_bias_frac) == 0:
                    nc.vector.tensor_scalar_add(o_slice, ps[:], b_t[:])
                else:
                    nc.scalar.add(o_slice, ps[:], b_t[:])

            def compute_seg(col0, nsub, xts):
                width = nsub * SUB
                o_t = op.tile([O, width], bf16, tag="o")
                per_store = max(1, nsub // store_split)
                if k_outer:
                    pss = [
                        pp.tile([O, SUB], fp32, tag="ps", name=f"ps_{col0}_{i}")
                        for i in range(nsub)
                    ]
                    for k in range(KT):
                        for s in range(nsub):
                            nc.tensor.matmul(
                                pss[s][:],
                                w_t[:, k * O : (k + 1) * O],
                                xts[k][:, s * SUB : (s + 1) * SUB],
                                start=(k == 0),
                                stop=(k == KT - 1),
                            )
                    for s in range(nsub):
                        bias_add(o_t[:, s * SUB : (s + 1) * SUB], pss[s], s)
                    nc.scalar.dma_start(out_d[:, col0 : col0 + width], o_t[:])
                    return
                stored = 0
                for s in range(nsub):
                    ps = pp.tile([O, SUB], fp32, tag="ps")
                    for k in range(KT):
                        nc.tensor.matmul(
                            ps[:],
                            w_t[:, k * O : (k + 1) * O],
                            xts[k][:, s * SUB : (s + 1) * SUB],
                            start=(k == 0),
                            stop=(k == KT - 1),
                        )
                    bias_add(o_t[:, s * SUB : (s + 1) * SUB], ps, s)
                    # store on the ACT HWDGE ring; loads use the SP ring
                    if s + 1 - stored >= per_store or s + 1 == nsub:
                        nc.scalar.dma_start(
                            out_d[:, col0 + stored * SUB : col0 + (s + 1) * SUB],
                            o_t[:, stored * SUB : (s + 1) * SUB],
                        )
                        stored = s + 1

            if repeats == 1:
                for si, (col0, nsub) in enumerate(segs):
                    xts = pending
                    if si + 1 < len(segs):
                        pending = load_seg(*segs[si + 1])
                    compute_seg(col0, nsub, xts)
            else:
                # hardware-loop variant for slope-based HW timing only
                compute_seg(*segs[0], pending)
                with tc.For_i(0, repeats, 1, hint_engines=(
                    mybir.EngineType.PE,
                    mybir.EngineType.Activation,
                    mybir.EngineType.SP,
                    mybir.EngineType.DVE,
                )):
                    for col0, nsub in segs:
                        xts = load_seg(col0, nsub)
                        compute_seg(col0, nsub, xts)
    nc.compile()
    return nc


def kernel(x, w_cls, b_cls, w_reg, b_reg, w_dir, b_dir):
    global LAST_RESULT, _nc_cache
    import ml_dtypes
    from concourse.bass_utils import run_bass_kernel_spmd

    bf = ml_dtypes.bfloat16
    e3 = ml_dtypes.float8_e3m4
    x = np.asarray(x, dtype=np.float32)
    w_all = np.concatenate(
        [np.asarray(w_cls), np.asarray(w_reg), np.asarray(w_dir)], axis=0
    ).astype(np.float32)  # [72, 384]
    lhsT = np.ascontiguousarray(w_all.T).astype(bf)  # [384, 72] bf16
    bias = (
        np.concatenate([np.asarray(b_cls), np.asarray(b_reg), np.asarray(b_dir)])
        .astype(np.float32)
        .reshape(O, 1)
    )

    xf = x.reshape(B, C, HWP)
    in_maps = []
    for core in range(NCORES):
        b_i, h_i = divmod(core, 2)
        shard = xf[b_i, :, h_i * HALF : (h_i + 1) * HALF].astype(e3)
        in_maps.append({"x": shard, "w": lhsT, "b": bias})

    if _nc_cache is None:
        _nc_cache = _build_nc()
    trace = os.environ.get("ANCHOR3D_TRACE", "0") == "1"
    LAST_RESULT = run_bass_kernel_spmd(
        _nc_cache, in_maps, core_ids=list(range(NCORES)), trace=trace
    )
    res = LAST_RESULT.results

    out = np.empty((B, O, HWP), dtype=np.float32)
    for core in range(NCORES):
        b_i, h_i = divmod(core, 2)
        out[b_i, :, h_i * HALF : (h_i + 1) * HALF] = res[core]["out"].astype(
            np.float32
        )
    out = out.reshape(B, O, H, W)
    return (
        np.ascontiguousarray(out[:, :O_CLS]),
        np.ascontiguousarray(out[:, O_CLS : O_CLS + O_REG]),
        np.ascontiguousarray(out[:, O_CLS + O_REG :]),
    )


# revision 33
# speedup vs baseline: 1.0406x; 1.0228x over previous
"""Anchor3DHead conv heads (three 1x1 convs) as a data-parallel Bass kernel.

Reference computes, for x [B=4, C=384, H=248, W=216]:
    cls  = einsum('bchw,oc->bohw', x, w_cls) + b_cls   # O=18
    reg  = einsum('bchw,oc->bohw', x, w_reg) + b_reg   # O=42
    dir  = einsum('bchw,oc->bohw', x, w_dir) + b_dir   # O=12

All three heads read the same x, so they are fused into a single GEMM with
O=72 output channels.  x[b] is already a contiguous [C, H*W] matrix, so the
whole problem is out[72, P] = W[72, 384] @ x[384, P] + b over P = B*H*W
pixels.  Sharding: pure data parallel over pixels — each of the 8 cores gets
half of one batch element ([384, 26784]); weights/bias are replicated.

The problem is memory-bound (headroom target_regime=memory): per-core HBM
traffic dominates, so input bytes are the whole game.  x is staged to the
device as fp8 E3M4 (TRN2-native FP8_EXP3, 4 mantissa bits); the TensorEngine
runs a mixed-dtype matmul (bf16 stationary weights x e3m4 moving operand —
verified bit-exact on HW) with fp32 PSUM accumulation.  The output is stored
bf16 and widened to f32 on the host.  Max rel err vs the f32 reference on
the real inputs is 1.46e-2, under the 2e-2 gate (bf16-x fallback: 4e-3).
"""

import os

import numpy as np

B, C, H, W = 4, 384, 248, 216
HWP = H * W            # 53568 pixels per batch element
HALF = HWP // 2        # 26784 pixels per core
O_CLS, O_REG, O_DIR = 18, 42, 12
O = O_CLS + O_REG + O_DIR  # 72 fused output channels
NCORES = 8
KT = C // 128          # 3 contraction tiles of 128
SUB = 496              # matmul free dim (<=512 fp32, 496*4B fits one PSUM bank)
NSUB = 4               # psum tiles per chunk
CHUNK = SUB * NSUB     # 2976 columns per DMA chunk
NCHUNK = HALF // CHUNK  # 9 chunks per core

LAST_RESULT = None  # BassKernelResults of the most recent run (for test.py)

_nc_cache = None


def _build_nc(
    repeats=1,
    xp_bufs=18,
    op_bufs=4,
    pp_bufs=8,
    store_split=1,
    dve_bias_frac=2,
    k_outer=False,
):
    import concourse.bacc as bacc
    import concourse.mybir as mybir
    import concourse.tile as tile

    nc = bacc.Bacc(
        "TRN2", target_bir_lowering=False, debug=False, num_devices=NCORES
    )
    fp32 = mybir.dt.float32
    bf16 = mybir.dt.bfloat16
    e3m4 = mybir.dt.float8e3
    x_d = nc.declare_dram_parameter("x", [C, HALF], e3m4, isOutput=False)
    w_d = nc.declare_dram_parameter("w", [C, O], bf16, isOutput=False)
    b_d = nc.declare_dram_parameter("b", [O, 1], fp32, isOutput=False)
    out_d = nc.declare_dram_parameter("out", [O, HALF], bf16, isOutput=True)

    # Column segments: full chunks first, then a finer-grained tail so the
    # last load->matmul->store dependency chain is short.
    segs = []
    col = 0
    for _ in range(NCHUNK - 1):
        segs.append((col, NSUB))
        col += CHUNK
    for nsub in (4, 2):
        segs.append((col, nsub))
        col += nsub * SUB
    assert col == HALF
    with tile.TileContext(nc) as tc:
        with (
            tc.tile_pool(name="wp", bufs=1) as wp,
            tc.tile_pool(name="xp", bufs=xp_bufs) as xp,
            tc.tile_pool(name="op", bufs=op_bufs) as op,
            tc.tile_pool(name="pp", bufs=pp_bufs, space="PSUM") as pp,
        ):
            # Stationary operand: w is already [K, M] = [384, 72] (lhsT).
            w_t = wp.tile([128, KT * O], bf16)
            b_t = wp.tile([O, 1], fp32)

            def load_seg(col0, nsub):
                width = nsub * SUB
                xts = []
                for k in range(KT):
                    xt = xp.tile([128, width], e3m4, tag="x")
                    nc.sync.dma_start(
                        xt[:], x_d[k * 128 : (k + 1) * 128, col0 : col0 + width]
                    )
                    xts.append(xt)
                return xts

            # first big loads go ahead of the tiny w/bias DMAs so the DMA
            # engines ramp to full rate immediately
            pending = load_seg(*segs[0])
            for k in range(KT):
                nc.sync.dma_start(
                    w_t[:, k * O : (k + 1) * O], w_d[k * 128 : (k + 1) * 128, :]
                )
            nc.sync.dma_start(b_t[:], b_d[:])

            def bias_add(o_slice, ps, s):
                # out = psum + bias (per-partition), PSUM -> SBUF; spread the
                # work across ACT and the otherwise-idle DVE
                if dve_bias_frac and (s % dve_bias_frac) == 0:
                    nc.vector.tensor_scalar_add(o_slice, ps[:], b_t[:])
                else:
                    nc.scalar.add(o_slice, ps[:], b_t[:])

            def compute_seg(col0, nsub, xts):
                width = nsub * SUB
                o_t = op.tile([O, width], bf16, tag="o")
                per_store = max(1, nsub // store_split)
                if k_outer:
                    pss = [
                        pp.tile([O, SUB], fp32, tag="ps", name=f"ps_{col0}_{i}")
                        for i in range(nsub)
                    ]
                    for k in range(KT):
                        for s in range(nsub):
                            nc.tensor.matmul(
                                pss[s][:],
                                w_t[:, k * O : (k + 1) * O],
                                xts[k][:, s * SUB : (s + 1) * SUB],
                                start=(k == 0),
                                stop=(k == KT - 1),
                            )
                    for s in range(nsub):
                        bias_add(o_t[:, s * SUB : (s + 1) * SUB], pss[s], s)
                    nc.scalar.dma_start(out_d[:, col0 : col0 + width], o_t[:])
                    return
                stored = 0
                for s in range(nsub):
                    ps = pp.tile([O, SUB], fp32, tag="ps")
                    for k in range(KT):
                        nc.tensor.matmul(
                            ps[:],
                            w_t[:, k * O : (k + 1) * O],
                            xts[k][:, s * SUB : (s + 1) * SUB],
                            start=(k == 0),
                            stop=(k == KT - 1),
                        )
                    bias_add(o_t[:, s * SUB : (s + 1) * SUB], ps, s)
                    # store on the ACT HWDGE ring; loads use the SP ring
                    if s + 1 - stored >= per_store or s + 1 == nsub:
                        nc.scalar.dma_start(
                            out_d[:, col0 + stored * SUB : col0 + (s + 1) * SUB],
                            o_t[:, stored * SUB : (s + 1) * SUB],
                        )
                        stored = s + 1

            if repeats == 1:
                for si, (col0, nsub) in enumerate(segs):
                    xts = pending
                    if si + 1 < len(segs):
                        pending = load_seg(*segs[si + 1])
                    compute_seg(col0, nsub, xts)
            else:
                # hardware-loop variant for slope-based HW timing only
                compute_seg(*segs[0], pending)
                with tc.For_i(0, repeats, 1, hint_engines=(
                    mybir.EngineType.PE,
                    mybir.EngineType.Activation,
                    mybir.EngineType.SP,
                    mybir.EngineType.DVE,
                )):
                    for col0, nsub in segs:
                        xts = load_seg(col0, nsub)
                        compute_seg(col0, nsub, xts)
    nc.compile()
    return nc


def kernel(x, w_cls, b_cls, w_reg, b_reg, w_dir, b_dir):
    global LAST_RESULT, _nc_cache
    import ml_dtypes
    from concourse.bass_utils import run_bass_kernel_spmd

    bf = ml_dtypes.bfloat16
    e3 = ml_dtypes.float8_e3m4
    x = np.asarray(x, dtype=np.float32)
    w_all = np.concatenate(
        [np.asarray(w_cls), np.asarray(w_reg), np.asarray(w_dir)], axis=0
    ).astype(np.float32)  # [72, 384]
    lhsT = np.ascontiguousarray(w_all.T).astype(bf)  # [384, 72] bf16
    bias = (
        np.concatenate([np.asarray(b_cls), np.asarray(b_reg), np.asarray(b_dir)])
        .astype(np.float32)
        .reshape(O, 1)
    )

    xf = x.reshape(B, C, HWP)
    in_maps = []
    for core in range(NCORES):
        b_i, h_i = divmod(core, 2)
        shard = xf[b_i, :, h_i * HALF : (h_i + 1) * HALF].astype(e3)
        in_maps.append({"x": shard, "w": lhsT, "b": bias})

    if _nc_cache is None:
        _nc_cache = _build_nc()
    trace = os.environ.get("ANCHOR3D_TRACE", "0") == "1"
    LAST_RESULT = run_bass_kernel_spmd(
        _nc_cache, in_maps, core_ids=list(range(NCORES)), trace=trace
    )
    res = LAST_RESULT.results

    out = np.empty((B, O, HWP), dtype=np.float32)
    for core in range(NCORES):
        b_i, h_i = divmod(core, 2)
        out[b_i, :, h_i * HALF : (h_i + 1) * HALF] = res[core]["out"].astype(
            np.float32
        )
    out = out.reshape(B, O, H, W)
    return (
        np.ascontiguousarray(out[:, :O_CLS]),
        np.ascontiguousarray(out[:, O_CLS : O_CLS + O_REG]),
        np.ascontiguousarray(out[:, O_CLS + O_REG :]),
    )
